# revision 6
# baseline (speedup 1.0000x reference)
"""Trainium2 Bass kernel for nn_MGVDModel (4-branch GNN: 2x GIN + 2x GAT).

8-way node sharding; 4 sequential SPMD launches on 8 NeuronCores:
  A: GIN conv1 (ast+dfg) + GAT score prepass (s1 = x @ (W1 a_src|a_dst))
  B: GAT conv1 (cfg+pdg) + GIN conv2 + GIN mean-pool
  C: GAT conv2 + GAT mean-pool
  D: sum pool partials + SE block + classifier MLP
Edge aggregation: host sorts edges by dst into 128-node windows; K chunks of
128 slots per window (src-table halves lo/hi since dma_gather idx is int16);
device gathers rows via batched dma_gather, builds one-hot selectors with
tensor_scalar(is_equal) on an iota tile, aggregates with TensorE matmuls in
PSUM. GAT softmax: unnormalized exp (safe: |scores| < ~3), denominator via
extra stream columns, self-loops as a contiguous "self chunk". Host moves
only index metadata and device-produced tensors between launches.
"""
import sys, os, time
sys.path.insert(0, '/opt/trn_rl_repo')
import numpy as np
import ml_dtypes

import concourse.bass as bass
import concourse.bacc as bacc
import concourse.mybir as mybir
import concourse.tile as tile
from concourse.bass_utils import run_bass_kernel_spmd

dt = mybir.dt
P = 128
N = 50000
G = 256
D = 64
HID = 128
HEADS = 4
NCLS = 7
NCORES = 8
SHARD = N // NCORES            # 6250
WPS = (SHARD + P - 1) // P     # 49
LASTW = SHARD - (WPS - 1) * P  # 106
KGIN = 6                       # 3 lo + 3 hi
KGAT = 7                       # 3 lo + 3 hi + self
NLO = 3
GRP = 7                        # windows per batched gather (49 = 7*7)
HALF = N // 2

bf16 = ml_dtypes.bfloat16
_cache = {}
AF = mybir.ActivationFunctionType
ALU = mybir.AluOpType


# ================================================================ host prep

def _wrap_idx(v):
    n = len(v)
    w = v.reshape(n // 16, 16).T.astype(np.int16)
    return np.ascontiguousarray(np.tile(w, (8, 1)))


def _build_edge_plan(ei, with_self):
    src, dst = np.asarray(ei[0]), np.asarray(ei[1])
    order = np.argsort(dst, kind='stable')
    src_s, dst_s = src[order], dst[order]
    K = KGAT if with_self else KGIN
    Kg = K - 1 if with_self else K
    nhi = Kg - NLO
    cores = []
    for c in range(NCORES):
        base = c * SHARD
        a = np.searchsorted(dst_s, base)
        b = np.searchsorted(dst_s, base + SHARD)
        es, ed, eo = src_s[a:b], dst_s[a:b], order[a:b]
        lo_cols, hi_cols, ld_cols, ss_cols, sd_cols = [], [], [], [], []
        for w in range(WPS):
            w0 = base + w * P
            w1 = min(w0 + P, base + SHARD)
            wa, wb = np.searchsorted(ed, w0), np.searchsorted(ed, w1)
            ws, wd = es[wa:wb], ed[wa:wb]
            is_lo = ws < HALF
            for sel, nchunk, sink, shift in ((is_lo, NLO, lo_cols, 0),
                                             (~is_lo, nhi, hi_cols, HALF)):
                hs, hd = ws[sel], wd[sel]
                cap = nchunk * P
                if len(hs) > cap:
                    raise RuntimeError(f"chunk overflow {len(hs)} > {cap}")
                pad = cap - len(hs)
                sink.append(np.concatenate([hs - shift, np.zeros(pad, np.int64)]))
                ld = np.concatenate([hd - w0, np.full(pad, 255, np.int64)])
                sv = np.concatenate([hs, np.zeros(pad, np.int64)])
                dv = np.concatenate([hd, np.zeros(pad, np.int64)])
                for k in range(nchunk):
                    ld_cols.append(ld[k * P:(k + 1) * P])
                    ss_cols.append(sv[k * P:(k + 1) * P])
                    sd_cols.append(dv[k * P:(k + 1) * P])
            if with_self:
                nreal = w1 - w0
                ld = np.arange(P)
                nodes = np.minimum(w0 + np.arange(P), N - 1)
                ld_cols.append(ld)
                ss_cols.append(nodes)
                sd_cols.append(nodes)
        cores.append(dict(
            lo=_wrap_idx(np.concatenate(lo_cols).astype(np.int16)),
            hi=_wrap_idx(np.concatenate(hi_cols).astype(np.int16)),
            ldst=np.stack(ld_cols, 1).astype(np.float32),
            ssrc=np.stack(ss_cols, 1),
            sdst=np.stack(sd_cols, 1),
            valid=np.stack(ld_cols, 1) != 255,
        ))
    return cores


def _pool_B(batch):
    batch = np.asarray(batch)
    counts = np.bincount(batch, minlength=G).astype(np.float64)
    inv = (1.0 / np.maximum(counts, 1.0)).astype(np.float32)
    outs, gbases = [], []
    for c in range(NCORES):
        base = c * SHARD
        gb = int(batch[base])
        gbases.append(gb)
        Bm = np.zeros((P, WPS * P), np.float32)
        for w in range(WPS):
            n0 = base + w * P
            n1 = min(n0 + P, base + SHARD)
            gl = batch[n0:n1] - gb
            Bm[np.arange(n1 - n0), w * P + gl] = inv[batch[n0:n1]]
        outs.append(Bm.astype(bf16))
    return outs, gbases


def _kblock(Wf, K, Dout):
    """lhsT [K, Dout] -> [128, (K//128)*Dout] with block kb at cols kb*Dout."""
    nb = K // P
    out = np.zeros((P, nb * Dout), np.float32)
    for kb in range(nb):
        out[:, kb * Dout:(kb + 1) * Dout] = Wf[kb * P:(kb + 1) * P, :]
    return out.astype(bf16)


def _fcol(v):
    return np.ascontiguousarray(np.asarray(v, np.float32).reshape(-1, 1))


def _bblock(b):
    b = np.asarray(b, np.float32).reshape(-1)
    nb = (len(b) + P - 1) // P
    out = np.zeros((P, nb), np.float32)
    for kb in range(nb):
        seg = b[kb * P:(kb + 1) * P]
        out[:len(seg), kb] = seg
    return out


def _prep_params(params):
    p = {}
    for g in ("ast", "dfg"):
        for li, cn in ((1, "c1"), (2, "c2")):
            cp = params[g][cn]
            p[f"{g}_w1_{li}"] = np.asarray(cp["w1"], np.float32).astype(bf16)
            p[f"{g}_b1_{li}"] = _fcol(cp["b1"])
            p[f"{g}_w2_{li}"] = np.asarray(cp["w2"], np.float32).astype(bf16)
            p[f"{g}_b2_{li}"] = _fcol(cp["b2"])
    for g in ("cfg", "pdg"):
        for li, cn in ((1, "c1"), (2, "c2")):
            cp = params[g][cn]
            W = np.asarray(cp["W"], np.float32)
            din = W.shape[0]
            a_src = np.asarray(cp["att_src"], np.float32)
            a_dst = np.asarray(cp["att_dst"], np.float32)
            v_src = np.stack([W[:, h * HID:(h + 1) * HID] @ a_src[h] for h in range(HEADS)], 1)
            v_dst = np.stack([W[:, h * HID:(h + 1) * HID] @ a_dst[h] for h in range(HEADS)], 1)
            p[f"{g}_v_{li}"] = np.concatenate([v_src, v_dst], 1).astype(bf16)
            Wst = (W.reshape(din, HEADS, HID).transpose(1, 0, 2)
                   .reshape(HEADS * din, HID) / 4.0)
            p[f"{g}_wst_{li}"] = _kblock(Wst, HEADS * din, HID)
            p[f"{g}_b_{li}"] = _fcol(cp["b"])
    p["se1_w"] = _kblock(np.asarray(params["se1"]["w"], np.float32), 512, HID)
    p["se1_b"] = _bblock(params["se1"]["b"])
    p["se2_w"] = np.asarray(params["se2"]["w"], np.float32).astype(bf16)
    p["se2_b"] = _bblock(params["se2"]["b"])
    p["cl1_w"] = _kblock(np.asarray(params["cl1"]["w"], np.float32), 512, 256)
    p["cl1_b"] = _bblock(params["cl1"]["b"])
    p["cl2_w"] = _kblock(np.asarray(params["cl2"]["w"], np.float32), 256, HID)
    p["cl2_b"] = _bblock(params["cl2"]["b"])
    p["cl3_w"] = np.asarray(params["cl3"]["w"], np.float32).astype(bf16)
    p["cl3_b"] = _bblock(params["cl3"]["b"])
    return p


def _sval_stream(pl, s_full):
    sv, dv, valid = pl["ssrc"], pl["sdst"], pl["valid"]
    ncols = sv.shape[1]
    out = np.zeros((P, ncols, 8), np.float32)
    out[:, :, 0:4] = np.where(valid[:, :, None], s_full[sv, 0:4], 0.0)
    out[:, :, 4:8] = np.where(valid[:, :, None], s_full[dv, 4:8], 0.0)
    return np.ascontiguousarray(out.reshape(P, ncols * 8)).astype(bf16)


def _pad_table(x):
    t = np.zeros((N, HID), bf16)
    t[:, :D] = np.asarray(x, np.float32).astype(bf16)
    return t


# ============================================================ bass building

def _consts(nc, tc, cpool):
    iota = cpool.tile([P, P], dt.float32, tag="iota")
    nc.gpsimd.iota(iota[:], pattern=[[1, P]], base=0, channel_multiplier=0,
                   allow_small_or_imprecise_dtypes=True)
    pidx = cpool.tile([P, 1], dt.float32, tag="pidx")
    nc.gpsimd.iota(pidx[:], pattern=[[0, 1]], base=0, channel_multiplier=1,
                   allow_small_or_imprecise_dtypes=True)
    ident = cpool.tile([P, P], dt.bfloat16, tag="ident")
    nc.vector.tensor_scalar(out=ident[:], in0=iota[:], scalar1=pidx[:, 0:1],
                            scalar2=None, op0=ALU.is_equal)
    return iota, ident


def _tr(nc, ps, sb, src_ap, rows, cols, ident, out_dtype=dt.bfloat16, tag="tp"):
    """TensorE transpose: src [rows, cols] -> returns sbuf tile [cols, rows]."""
    pt = ps.tile([P, P], dt.bfloat16, tag=tag)
    nc.tensor.transpose(pt[0:cols, 0:rows], src_ap, ident[0:rows, 0:rows])
    ot = sb.tile([P, P], out_dtype, tag=tag + "s")
    nc.scalar.copy(ot[0:cols, 0:rows], pt[0:cols, 0:rows])
    return ot


def _gather_grp(nc, tc, gsem, ss, dest, table, ilo, ihi, grp, npc):
    """Two batched dma_gathers for one window group. npc = chunks/side/window."""
    nidx = GRP * npc * P
    for it, lohi in ((ilo, 0), (ihi, 1)):
        base_ic = grp * (nidx // 16)
        with tc.tile_critical():
            nc.gpsimd.dma_gather(
                out_ap=dest[:, 0:GRP * npc, :] if lohi == 0 else dest[:, GRP * npc:, :],
                in_ap=table[0:HALF, :] if lohi == 0 else table[HALF:N, :],
                idxs_ap=it[:, base_ic:base_ic + nidx // 16],
                num_idxs=nidx, num_idxs_reg=nidx,
                elem_size=HID, single_packet=False,
            ).then_inc(gsem, 16)
            ss['v'] += 16
            nc.gpsimd.wait_ge(gsem, ss['v'])


def _col_of(wl, k, npc):
    return wl * npc + k if k < npc else GRP * npc + wl * npc + (k - npc)


def _io(nc, tensors):
    def inp(name, shape, dty):
        tensors[name] = nc.dram_tensor(name, shape, dty, kind="ExternalInput")
        return tensors[name]

    def outp(name, shape, dty):
        tensors[name] = nc.dram_tensor(name, shape, dty, kind="ExternalOutput")
        return tensors[name]
    return inp, outp


def _gin_layer(nc, tc, gsem, ss, t, g, F, cpool, sb, gp, ps, pagg, ppool,
               iota, ident, pool):
    """One GIN conv layer over this core's shard. F = input feat width."""
    w1 = cpool.tile([F, HID], dt.bfloat16, tag=f"gw1{g}")
    nc.sync.dma_start(w1[:], t[f"w1_{g}"][:])
    w2 = cpool.tile([HID, HID], dt.bfloat16, tag=f"gw2{g}")
    nc.sync.dma_start(w2[:], t[f"w2_{g}"][:])
    b1 = cpool.tile([HID, 1], dt.float32, tag=f"gb1{g}")
    nc.sync.dma_start(b1[:], t[f"b1_{g}"][:])
    b2 = cpool.tile([HID, 1], dt.float32, tag=f"gb2{g}")
    nc.sync.dma_start(b2[:], t[f"b2_{g}"][:])
    NCOL = WPS * KGIN
    ldt = cpool.tile([P, NCOL], dt.float32, tag=f"gld{g}")
    nc.sync.dma_start(ldt[:], t[f"ld_{g}"][:])
    ilo = cpool.tile([P, NCOL // 2 * P // 16], dt.int16, tag=f"gilo{g}")
    nc.sync.dma_start(ilo[:], t[f"ilo_{g}"][:])
    ihi = cpool.tile([P, NCOL // 2 * P // 16], dt.int16, tag=f"gihi{g}")
    nc.sync.dma_start(ihi[:], t[f"ihi_{g}"][:])
    if pool:
        Bw = cpool.tile([P, WPS * P], dt.bfloat16, tag=f"gBw{g}")
        nc.sync.dma_start(Bw[:], t[f"Bw_{g}"][:])
        poolps = ppool.tile([P, HID], dt.float32, tag=f"pool{g}")
    for grp in range(WPS // GRP):
        dest = gp.tile([P, GRP * KGIN, HID], dt.bfloat16, tag="gdest")
        _gather_grp(nc, tc, gsem, ss, dest, t[f"tab_{g}"], ilo, ihi, grp, NLO)
        for wl in range(GRP):
            w = grp * GRP + wl
            nreal = LASTW if w == WPS - 1 else P
            agg = pagg.tile([P, F], dt.float32, tag="agg")
            for k in range(KGIN):
                ci = w * KGIN + k
                S = sb.tile([P, P], dt.bfloat16, tag="S")
                nc.vector.tensor_scalar(out=S[:], in0=iota[:],
                                        scalar1=ldt[:, ci:ci + 1], scalar2=None,
                                        op0=ALU.is_equal)
                nc.tensor.matmul(agg[:], S[:], dest[:, _col_of(wl, k, NLO), 0:F],
                                 start=(k == 0), stop=(k == KGIN - 1))
            xw = sb.tile([P, F], dt.bfloat16, tag="xw")
            nc.sync.dma_start(xw[0:nreal, :], t[f"hs_{g}"][w * P:w * P + nreal, 0:F])
            hpre = sb.tile([P, F], dt.bfloat16, tag="hpre")
            if nreal < P:
                nc.vector.memset(hpre[:], 0.0)
            nc.vector.tensor_add(hpre[0:nreal, :], agg[0:nreal, :], xw[0:nreal, :])
            hT = _tr(nc, ps, sb, hpre[0:P, 0:F], P, F, ident)
            hps = ps.tile([HID, P], dt.float32, tag="hps")
            nc.tensor.matmul(hps[:], w1[:], hT[0:F, 0:P], start=True, stop=True)
            hr = sb.tile([HID, P], dt.bfloat16, tag="hr")
            nc.scalar.activation(hr[:], hps[:], AF.Relu, bias=b1[:, 0:1])
            h2ps = ps.tile([HID, P], dt.float32, tag="hps")
            nc.tensor.matmul(h2ps[:], w2[:], hr[:], start=True, stop=True)
            h2 = sb.tile([HID, P], dt.bfloat16, tag="h2")
            nc.scalar.activation(h2[:], h2ps[:], AF.Relu, bias=b2[:, 0:1])
            h2T = _tr(nc, ps, sb, h2[0:HID, 0:P], HID, P, ident)
            if pool:
                nc.tensor.matmul(poolps[:], Bw[:, w * P:(w + 1) * P], h2T[0:P, 0:HID],
                                 start=(w == 0), stop=(w == WPS - 1),
                                 skip_group_check=True)
            else:
                nc.sync.dma_start(t[f"h1_{g}"][w * P:w * P + nreal, :],
                                  h2T[0:nreal, 0:HID])
    if pool:
        pout = sb.tile([P, HID], dt.float32, tag="pout")
        nc.vector.tensor_copy(pout[:], poolps[:])
        nc.sync.dma_start(t[f"pool_{g}"][:], pout[:])


def _gat_layer(nc, tc, gsem, ss, t, g, layer, cpool, sb, gp, ps, pagg, pden,
               ppool, iota, ident):
    F = D if layer == 1 else HID
    KB = HEADS * F // P           # k-blocks for Wstack (2 or 4)
    NCOL = WPS * KGAT
    NGC = WPS * (KGAT - 1)
    wst = cpool.tile([P, KB * HID], dt.bfloat16, tag=f"awst{g}")
    nc.sync.dma_start(wst[:], t[f"wst_{g}"][:])
    bb = cpool.tile([HID, 1], dt.float32, tag=f"ab{g}")
    nc.sync.dma_start(bb[:], t[f"b_{g}"][:])
    ldt = cpool.tile([P, NCOL], dt.float32, tag=f"ald{g}")
    nc.sync.dma_start(ldt[:], t[f"ld_{g}"][:])
    svt = cpool.tile([P, NCOL * 8], dt.bfloat16, tag=f"asv{g}")
    nc.sync.dma_start(svt[:], t[f"sv_{g}"][:])
    ilo = cpool.tile([P, NGC // 2 * P // 16], dt.int16, tag=f"ailo{g}")
    nc.sync.dma_start(ilo[:], t[f"ilo_{g}"][:])
    ihi = cpool.tile([P, NGC // 2 * P // 16], dt.int16, tag=f"aihi{g}")
    nc.sync.dma_start(ihi[:], t[f"ihi_{g}"][:])
    if layer == 1:
        v2 = cpool.tile([HID, 8], dt.bfloat16, tag=f"av2{g}")
        nc.sync.dma_start(v2[:], t[f"v2_{g}"][:])
    else:
        Bw = cpool.tile([P, WPS * P], dt.bfloat16, tag=f"aBw{g}")
        nc.sync.dma_start(Bw[:], t[f"Bw_{g}"][:])
        poolps = ppool.tile([P, HID], dt.float32, tag=f"pool{g}")
    svv = svt[:].rearrange("p (c e) -> p c e", e=8)
    SW = HEADS * F + (4 if layer == 1 else 0)   # stream width (den cols in l1)
    for grp in range(WPS // GRP):
        dest = gp.tile([P, GRP * (KGAT - 1), HID], dt.bfloat16, tag="gdest")
        _gather_grp(nc, tc, gsem, ss, dest, t[f"tab_{g}"], ilo, ihi, grp, NLO)
        for wl in range(GRP):
            w = grp * GRP + wl
            nreal = LASTW if w == WPS - 1 else P
            # batched e-path for the window's 7 chunks
            a = sb.tile([P, KGAT * 4], dt.float32, tag="a")
            nc.vector.tensor_tensor(out=a[:], in0=svv[:, w * KGAT:(w + 1) * KGAT, 0:4],
                                    in1=svv[:, w * KGAT:(w + 1) * KGAT, 4:8],
                                    op=ALU.add)
            lr = sb.tile([P, KGAT * 4], dt.float32, tag="lr")
            nc.scalar.activation(lr[:], a[:], AF.Lrelu, alpha=0.2)
            ev = sb.tile([P, KGAT * 4], dt.float32, tag="ev")
            nc.scalar.activation(ev[:], lr[:], AF.Exp)
            evb = sb.tile([P, KGAT * 4], dt.bfloat16, tag="evb")
            nc.vector.tensor_copy(evb[:], ev[:])
            agg = pagg.tile([P, SW], dt.float32, tag="agg")
            if layer == 2:
                den = pden.tile([P, 4], dt.float32, tag="den")
            selfw = sb.tile([P, HID], dt.bfloat16, tag="selfw")
            if nreal < P:
                nc.vector.memset(selfw[:], 0.0)
            nc.sync.dma_start(selfw[0:nreal, :], t[f"hs_{g}"][w * P:w * P + nreal, :])
            for k in range(KGAT):
                ci = w * KGAT + k
                rhs_x = (dest[:, _col_of(wl, k, NLO), 0:F] if k < KGAT - 1
                         else selfw[:, 0:F])
                stream = sb.tile([P, SW], dt.bfloat16, tag="stream")
                for h in range(HEADS):
                    nc.scalar.activation(stream[:, h * F:(h + 1) * F], rhs_x,
                                         AF.Copy, scale=ev[:, k * 4 + h:k * 4 + h + 1])
                if layer == 1:
                    nc.vector.tensor_copy(stream[:, HEADS * F:SW],
                                          evb[:, k * 4:(k + 1) * 4])
                S = sb.tile([P, P], dt.bfloat16, tag="S")
                nc.vector.tensor_scalar(out=S[:], in0=iota[:],
                                        scalar1=ldt[:, ci:ci + 1], scalar2=None,
                                        op0=ALU.is_equal)
                nc.tensor.matmul(agg[:], S[:], stream[:],
                                 start=(k == 0), stop=(k == KGAT - 1))
                if layer == 2:
                    nc.tensor.matmul(den[:], S[:], evb[:, k * 4:(k + 1) * 4],
                                     start=(k == 0), stop=(k == KGAT - 1))
            recip = sb.tile([P, 4], dt.float32, tag="recip")
            nc.vector.reciprocal(recip[:], agg[:, HEADS * F:SW] if layer == 1 else den[:])
            norm = sb.tile([P, HEADS * F], dt.bfloat16, tag="norm")
            for h in range(HEADS):
                nc.scalar.activation(norm[:, h * F:(h + 1) * F],
                                     agg[:, h * F:(h + 1) * F], AF.Copy,
                                     scale=recip[:, h:h + 1])
            hps = ps.tile([HID, P], dt.float32, tag="hps")
            for kb in range(KB):
                nT = _tr(nc, ps, sb, norm[0:P, kb * P:(kb + 1) * P], P, P, ident)
                nc.tensor.matmul(hps[:], wst[:, kb * HID:(kb + 1) * HID], nT[0:P, 0:P],
                                 start=(kb == 0), stop=(kb == KB - 1))
            h1 = sb.tile([HID, P], dt.bfloat16, tag="h1")
            nc.scalar.activation(h1[:], hps[:], AF.Relu, bias=bb[:, 0:1])
            h1T = _tr(nc, ps, sb, h1[0:HID, 0:P], HID, P, ident)
            if layer == 1:
                sps = ps.tile([16, P], dt.float32, tag="tp")
                nc.tensor.matmul(sps[0:8, :], v2[:], h1[:, :], start=True, stop=True)
                scp = sb.tile([16, P], dt.bfloat16, tag="scp")
                nc.vector.memset(scp[:], 0.0)
                nc.scalar.copy(scp[0:8, :], sps[0:8, :])
                sT = _tr(nc, ps, sb, scp[0:16, 0:P], 16, P, ident, dt.float32)
                nc.sync.dma_start(t[f"s2_{g}"][w * P:w * P + nreal, :],
                                  sT[0:nreal, 0:8])
                nc.sync.dma_start(t[f"h1_{g}"][w * P:w * P + nreal, :],
                                  h1T[0:nreal, 0:HID])
            else:
                nc.tensor.matmul(poolps[:], Bw[:, w * P:(w + 1) * P], h1T[0:P, 0:HID],
                                 start=(w == 0), stop=(w == WPS - 1),
                                 skip_group_check=True)
    if layer == 2:
        pout = sb.tile([P, HID], dt.float32, tag="pout")
        nc.vector.tensor_copy(pout[:], poolps[:])
        nc.sync.dma_start(t[f"pool_{g}"][:], pout[:])


def build_A():
    nc = bacc.Bacc("TRN2")
    t = {}
    inp, outp = _io(nc, t)
    NCOL = WPS * KGIN
    for g in ("ast", "dfg"):
        inp(f"tab_{g}", [N, HID], dt.bfloat16)
        inp(f"hs_{g}", [SHARD, HID], dt.bfloat16)
        inp(f"ilo_{g}", [P, NCOL // 2 * P // 16], dt.int16)
        inp(f"ihi_{g}", [P, NCOL // 2 * P // 16], dt.int16)
        inp(f"ld_{g}", [P, NCOL], dt.float32)
        inp(f"w1_{g}", [D, HID], dt.bfloat16)
        inp(f"b1_{g}", [HID, 1], dt.float32)
        inp(f"w2_{g}", [HID, HID], dt.bfloat16)
        inp(f"b2_{g}", [HID, 1], dt.float32)
        outp(f"h1_{g}", [SHARD, HID], dt.bfloat16)
    for g in ("cfg", "pdg"):
        inp(f"xs_{g}", [SHARD, HID], dt.bfloat16)
        inp(f"v_{g}", [D, 8], dt.bfloat16)
        outp(f"s1_{g}", [SHARD, 8], dt.float32)
    with tile.TileContext(nc) as tc:
        gsem = nc.alloc_semaphore("gsem")
        ss = {'v': 0}
        with (
            tc.tile_pool(name="const", bufs=1) as cpool,
            tc.tile_pool(name="sb", bufs=3) as sb,
            tc.tile_pool(name="gat", bufs=2) as gp,
            tc.tile_pool(name="ps", bufs=2, space="PSUM") as ps,
            tc.tile_pool(name="pagg", bufs=2, space="PSUM") as pagg,
            tc.tile_pool(name="ppool", bufs=1, space="PSUM") as ppool,
        ):
            iota, ident = _consts(nc, tc, cpool)
            for g in ("ast", "dfg"):
                _gin_layer(nc, tc, gsem, ss, t, g, D, cpool, sb, gp, ps, pagg,
                           ppool, iota, ident, pool=False)
            for g in ("cfg", "pdg"):
                v = cpool.tile([D, 8], dt.bfloat16, tag=f"v{g}")
                nc.sync.dma_start(v[:], t[f"v_{g}"][:])
                for w in range(WPS):
                    nreal = LASTW if w == WPS - 1 else P
                    xw = sb.tile([P, D], dt.bfloat16, tag="xw")
                    if nreal < P:
                        nc.vector.memset(xw[:], 0.0)
                    nc.sync.dma_start(xw[0:nreal, :],
                                      t[f"xs_{g}"][w * P:w * P + nreal, 0:D])
                    xT = _tr(nc, ps, sb, xw[0:P, 0:D], P, D, ident)
                    sps = ps.tile([16, P], dt.float32, tag="hps")
                    nc.tensor.matmul(sps[0:8, :], v[:], xT[0:D, 0:P],
                                     start=True, stop=True)
                    scp = sb.tile([16, P], dt.bfloat16, tag="scp")
                    nc.vector.memset(scp[:], 0.0)
                    nc.scalar.copy(scp[0:8, :], sps[0:8, :])
                    sT = _tr(nc, ps, sb, scp[0:16, 0:P], 16, P, ident, dt.float32)
                    nc.sync.dma_start(t[f"s1_{g}"][w * P:w * P + nreal, :],
                                      sT[0:nreal, 0:8])
    nc.compile()
    return nc, t


def build_B():
    nc = bacc.Bacc("TRN2")
    t = {}
    inp, outp = _io(nc, t)
    NCG = WPS * KGAT
    NCN = WPS * KGIN
    for g in ("cfg", "pdg"):
        inp(f"tab_{g}", [N, HID], dt.bfloat16)
        inp(f"hs_{g}", [SHARD, HID], dt.bfloat16)
        inp(f"ilo_{g}", [P, (NCG - WPS) // 2 * P // 16], dt.int16)
        inp(f"ihi_{g}", [P, (NCG - WPS) // 2 * P // 16], dt.int16)
        inp(f"ld_{g}", [P, NCG], dt.float32)
        inp(f"sv_{g}", [P, NCG * 8], dt.bfloat16)
        inp(f"wst_{g}", [P, 2 * HID], dt.bfloat16)
        inp(f"b_{g}", [HID, 1], dt.float32)
        inp(f"v2_{g}", [HID, 8], dt.bfloat16)
        outp(f"h1_{g}", [SHARD, HID], dt.bfloat16)
        outp(f"s2_{g}", [SHARD, 8], dt.float32)
    for g in ("ast", "dfg"):
        inp(f"tab_{g}", [N, HID], dt.bfloat16)
        inp(f"hs_{g}", [SHARD, HID], dt.bfloat16)
        inp(f"ilo_{g}", [P, NCN // 2 * P // 16], dt.int16)
        inp(f"ihi_{g}", [P, NCN // 2 * P // 16], dt.int16)
        inp(f"ld_{g}", [P, NCN], dt.float32)
        inp(f"w1_{g}", [HID, HID], dt.bfloat16)
        inp(f"b1_{g}", [HID, 1], dt.float32)
        inp(f"w2_{g}", [HID, HID], dt.bfloat16)
        inp(f"b2_{g}", [HID, 1], dt.float32)
        inp(f"Bw_{g}", [P, WPS * P], dt.bfloat16)
        outp(f"pool_{g}", [P, HID], dt.float32)
    with tile.TileContext(nc) as tc:
        gsem = nc.alloc_semaphore("gsem")
        ss = {'v': 0}
        with (
            tc.tile_pool(name="const", bufs=1) as cpool,
            tc.tile_pool(name="sb", bufs=3) as sb,
            tc.tile_pool(name="gat", bufs=2) as gp,
            tc.tile_pool(name="ps", bufs=2, space="PSUM") as ps,
            tc.tile_pool(name="pagg", bufs=2, space="PSUM") as pagg,
            tc.tile_pool(name="ppool", bufs=1, space="PSUM") as ppool,
        ):
            iota, ident = _consts(nc, tc, cpool)
            for g in ("cfg", "pdg"):
                _gat_layer(nc, tc, gsem, ss, t, g, 1, cpool, sb, gp, ps, pagg,
                           None, ppool, iota, ident)
            for g in ("ast", "dfg"):
                _gin_layer(nc, tc, gsem, ss, t, g, HID, cpool, sb, gp, ps, pagg,
                           ppool, iota, ident, pool=True)
    nc.compile()
    return nc, t


def build_C():
    nc = bacc.Bacc("TRN2")
    t = {}
    inp, outp = _io(nc, t)
    NCG = WPS * KGAT
    for g in ("cfg", "pdg"):
        inp(f"tab_{g}", [N, HID], dt.bfloat16)
        inp(f"hs_{g}", [SHARD, HID], dt.bfloat16)
        inp(f"ilo_{g}", [P, (NCG - WPS) // 2 * P // 16], dt.int16)
        inp(f"ihi_{g}", [P, (NCG - WPS) // 2 * P // 16], dt.int16)
        inp(f"ld_{g}", [P, NCG], dt.float32)
        inp(f"sv_{g}", [P, NCG * 8], dt.bfloat16)
        inp(f"wst_{g}", [P, 4 * HID], dt.bfloat16)
        inp(f"b_{g}", [HID, 1], dt.float32)
        inp(f"Bw_{g}", [P, WPS * P], dt.bfloat16)
        outp(f"pool_{g}", [P, HID], dt.float32)
    with tile.TileContext(nc) as tc:
        gsem = nc.alloc_semaphore("gsem")
        ss = {'v': 0}
        with (
            tc.tile_pool(name="const", bufs=1) as cpool,
            tc.tile_pool(name="sb", bufs=3) as sb,
            tc.tile_pool(name="gat", bufs=2) as gp,
            tc.tile_pool(name="ps", bufs=2, space="PSUM") as ps,
            tc.tile_pool(name="pagg", bufs=1, space="PSUM") as pagg,
            tc.tile_pool(name="pden", bufs=1, space="PSUM") as pden,
            tc.tile_pool(name="ppool", bufs=1, space="PSUM") as ppool,
        ):
            iota, ident = _consts(nc, tc, cpool)
            for g in ("cfg", "pdg"):
                _gat_layer(nc, tc, gsem, ss, t, g, 2, cpool, sb, gp, ps, pagg,
                           pden, ppool, iota, ident)
    nc.compile()
    return nc, t


def build_D():
    nc = bacc.Bacc("TRN2")
    t = {}
    inp, outp = _io(nc, t)
    inp("c_parts", [NCORES * 2, G, 512], dt.float32)
    inp("se1_w", [P, 4 * HID], dt.bfloat16); inp("se1_b", [P, 1], dt.float32)
    inp("se2_w", [HID, 512], dt.bfloat16); inp("se2_b", [P, 4], dt.float32)
    inp("cl1_w", [P, 4 * 256], dt.bfloat16); inp("cl1_b", [P, 2], dt.float32)
    inp("cl2_w", [P, 2 * HID], dt.bfloat16); inp("cl2_b", [P, 1], dt.float32)
    inp("cl3_w", [HID, NCLS], dt.bfloat16); inp("cl3_b", [P, 1], dt.float32)
    out = outp("out", [G, NCLS], dt.float32)
    with tile.TileContext(nc) as tc:
        with (
            tc.tile_pool(name="const", bufs=1) as cpool,
            tc.tile_pool(name="sb", bufs=2) as sb,
            tc.tile_pool(name="ps", bufs=1, space="PSUM") as ps,
            tc.tile_pool(name="ps2", bufs=2, space="PSUM") as ps2,
        ):
            iota, ident = _consts(nc, tc, cpool)
            w = {}
            for nm, shp in (("se1_w", [P, 4 * HID]), ("se2_w", [HID, 512]),
                            ("cl1_w", [P, 4 * 256]), ("cl2_w", [P, 2 * HID]),
                            ("cl3_w", [HID, NCLS])):
                w[nm] = cpool.tile(shp, dt.bfloat16, tag=nm, name=nm)
                nc.sync.dma_start(w[nm][:], t[nm][:])
            for nm, shp in (("se1_b", [P, 1]), ("se2_b", [P, 4]),
                            ("cl1_b", [P, 2]), ("cl2_b", [P, 1]), ("cl3_b", [P, 1])):
                w[nm] = cpool.tile(shp, dt.float32, tag=nm, name=nm)
                nc.sync.dma_start(w[nm][:], t[nm][:])
            c_t = []
            for rt in range(2):
                acc = sb.tile([P, 512], dt.float32, tag=f"acc{rt}")
                pt0 = sb.tile([P, 512], dt.float32, tag="cpart")
                nc.sync.dma_start(pt0[:], t["c_parts"][0, rt * P:(rt + 1) * P, :])
                nc.vector.tensor_copy(acc[:], pt0[:])
                for i in range(1, NCORES * 2):
                    pt = sb.tile([P, 512], dt.float32, tag="cpart")
                    nc.sync.dma_start(pt[:], t["c_parts"][i, rt * P:(rt + 1) * P, :])
                    nc.vector.tensor_add(acc[:], acc[:], pt[:])
                c_t.append(acc)
            cT, cTb = [], []
            for kb in range(4):
                tt = sb.tile([P, G], dt.float32, tag=f"cT{kb}")
                for rt in range(2):
                    cb = sb.tile([P, P], dt.bfloat16, tag="cb")
                    nc.vector.tensor_copy(cb[:], c_t[rt][:, kb * P:(kb + 1) * P])
                    pt = ps2.tile([P, P], dt.bfloat16, tag="tp")
                    nc.tensor.transpose(pt[:], cb[:], ident[:])
                    nc.vector.tensor_copy(tt[:, rt * P:(rt + 1) * P], pt[:])
                cT.append(tt)
                bbt = sb.tile([P, G], dt.bfloat16, tag=f"cTb{kb}")
                nc.vector.tensor_copy(bbt[:], tt[:])
                cTb.append(bbt)
            sps = ps.tile([P, G], dt.float32, tag="sps")
            for kb in range(4):
                nc.tensor.matmul(sps[:], w["se1_w"][:, kb * HID:(kb + 1) * HID],
                                 cTb[kb][:], start=(kb == 0), stop=(kb == 3))
            ser = sb.tile([P, G], dt.bfloat16, tag="ser")
            nc.scalar.activation(ser[:], sps[:], AF.Relu, bias=w["se1_b"][:, 0:1])
            gate = []
            for kb in range(4):
                gps = ps2.tile([P, G], dt.float32, tag="gps")
                nc.tensor.matmul(gps[:], w["se2_w"][:, kb * P:(kb + 1) * P], ser[:],
                                 start=True, stop=True)
                sg = sb.tile([P, G], dt.float32, tag="sg")
                nc.scalar.activation(sg[:], gps[:], AF.Sigmoid,
                                     bias=w["se2_b"][:, kb:kb + 1])
                gt = sb.tile([P, G], dt.bfloat16, tag=f"gt{kb}")
                nc.vector.tensor_mul(gt[:], cT[kb][:], sg[:])
                gate.append(gt)
            h1t = []
            for mt in range(2):
                hp = ps2.tile([P, G], dt.float32, tag="hp")
                for kb in range(4):
                    nc.tensor.matmul(hp[:], w["cl1_w"][:, kb * 256 + mt * P:
                                                       kb * 256 + (mt + 1) * P],
                                     gate[kb][:], start=(kb == 0), stop=(kb == 3))
                hr = sb.tile([P, G], dt.bfloat16, tag=f"hr{mt}")
                nc.scalar.activation(hr[:], hp[:], AF.Relu, bias=w["cl1_b"][:, mt:mt + 1])
                h1t.append(hr)
            hp2 = ps2.tile([P, G], dt.float32, tag="hp")
            for mt in range(2):
                nc.tensor.matmul(hp2[:], w["cl2_w"][:, mt * HID:(mt + 1) * HID],
                                 h1t[mt][:], start=(mt == 0), stop=(mt == 1))
            hr2 = sb.tile([P, G], dt.bfloat16, tag="hr2")
            nc.scalar.activation(hr2[:], hp2[:], AF.Relu, bias=w["cl2_b"][:, 0:1])
            op = ps2.tile([16, G], dt.float32, tag="tp")
            nc.tensor.matmul(op[0:NCLS, :], w["cl3_w"][:], hr2[:], start=True, stop=True)
            ob = sb.tile([16, G], dt.bfloat16, tag="ob")
            nc.vector.memset(ob[:], 0.0)
            nc.scalar.activation(ob[0:NCLS, :], op[0:NCLS, :], AF.Identity,
                                 bias=w["cl3_b"][0:NCLS, 0:1])
            for rt in range(2):
                pt = ps2.tile([P, 16], dt.bfloat16, tag="tp")
                nc.tensor.transpose(pt[:, 0:16], ob[0:16, rt * P:(rt + 1) * P],
                                    ident[0:16, 0:16])
                of = sb.tile([P, 16], dt.float32, tag="of")
                nc.vector.tensor_copy(of[:, 0:16], pt[:, 0:16])
                nc.sync.dma_start(out[rt * P:(rt + 1) * P, :], of[:, 0:NCLS])
    nc.compile()
    return nc, t


# ================================================================== runner

def kernel(**inputs):
    params = inputs["params"]
    pp = _prep_params(params)
    graphs = {
        "ast": ("gin", inputs["ast_x"], inputs["ast_ei"], inputs["batch_ast"]),
        "dfg": ("gin", inputs["dfg_x"], inputs["dfg_ei"], inputs["batch_dfg"]),
        "cfg": ("gat", inputs["cfg_x"], inputs["cfg_ei"], inputs["batch_cfg"]),
        "pdg": ("gat", inputs["pdg_x"], inputs["pdg_ei"], inputs["batch_pdg"]),
    }
    plans = {g: _build_edge_plan(np.asarray(v[2]), v[0] == "gat")
             for g, v in graphs.items()}
    pools = {g: _pool_B(v[3]) for g, v in graphs.items()}
    tabs = {g: _pad_table(v[1]) for g, v in graphs.items()}
    walls = {}

    if "A" not in _cache:
        _cache["A"] = build_A()
    ncA, tA = _cache["A"]
    in_maps = []
    for c in range(NCORES):
        m = {}
        for g in ("ast", "dfg"):
            pl = plans[g][c]
            m[f"tab_{g}"] = tabs[g]
            m[f"hs_{g}"] = tabs[g][c * SHARD:(c + 1) * SHARD]
            m[f"ilo_{g}"], m[f"ihi_{g}"], m[f"ld_{g}"] = pl["lo"], pl["hi"], pl["ldst"]
            m[f"w1_{g}"] = pp[f"{g}_w1_1"]; m[f"b1_{g}"] = pp[f"{g}_b1_1"]
            m[f"w2_{g}"] = pp[f"{g}_w2_1"]; m[f"b2_{g}"] = pp[f"{g}_b2_1"]
        for g in ("cfg", "pdg"):
            m[f"xs_{g}"] = tabs[g][c * SHARD:(c + 1) * SHARD]
            m[f"v_{g}"] = pp[f"{g}_v_1"]
        in_maps.append(m)
    t0 = time.time()
    resA = run_bass_kernel_spmd(ncA, in_maps, list(range(NCORES))).results
    walls["A"] = time.time() - t0
    h1gin = {g: np.ascontiguousarray(
        np.concatenate([resA[c][f"h1_{g}"] for c in range(NCORES)]))
        for g in ("ast", "dfg")}
    s1 = {g: np.concatenate([resA[c][f"s1_{g}"] for c in range(NCORES)]).astype(np.float32)
          for g in ("cfg", "pdg")}

    if "B" not in _cache:
        _cache["B"] = build_B()
    ncB, tB = _cache["B"]
    in_maps = []
    for c in range(NCORES):
        m = {}
        for g in ("cfg", "pdg"):
            pl = plans[g][c]
            m[f"tab_{g}"] = tabs[g]
            m[f"hs_{g}"] = tabs[g][c * SHARD:(c + 1) * SHARD]
            m[f"ilo_{g}"], m[f"ihi_{g}"], m[f"ld_{g}"] = pl["lo"], pl["hi"], pl["ldst"]
            m[f"sv_{g}"] = _sval_stream(pl, s1[g])
            m[f"wst_{g}"] = pp[f"{g}_wst_1"]; m[f"b_{g}"] = pp[f"{g}_b_1"]
            m[f"v2_{g}"] = pp[f"{g}_v_2"]
        for g in ("ast", "dfg"):
            pl = plans[g][c]
            m[f"tab_{g}"] = h1gin[g]
            m[f"hs_{g}"] = h1gin[g][c * SHARD:(c + 1) * SHARD]
            m[f"ilo_{g}"], m[f"ihi_{g}"], m[f"ld_{g}"] = pl["lo"], pl["hi"], pl["ldst"]
            m[f"w1_{g}"] = pp[f"{g}_w1_2"]; m[f"b1_{g}"] = pp[f"{g}_b1_2"]
            m[f"w2_{g}"] = pp[f"{g}_w2_2"]; m[f"b2_{g}"] = pp[f"{g}_b2_2"]
            m[f"Bw_{g}"] = pools[g][0][c]
        in_maps.append(m)
    t0 = time.time()
    resB = run_bass_kernel_spmd(ncB, in_maps, list(range(NCORES))).results
    walls["B"] = time.time() - t0
    h1gat = {g: np.ascontiguousarray(
        np.concatenate([resB[c][f"h1_{g}"] for c in range(NCORES)]))
        for g in ("cfg", "pdg")}
    s2 = {g: np.concatenate([resB[c][f"s2_{g}"] for c in range(NCORES)]).astype(np.float32)
          for g in ("cfg", "pdg")}

    if "C" not in _cache:
        _cache["C"] = build_C()
    ncC, tC = _cache["C"]
    in_maps = []
    for c in range(NCORES):
        m = {}
        for g in ("cfg", "pdg"):
            pl = plans[g][c]
            m[f"tab_{g}"] = h1gat[g]
            m[f"hs_{g}"] = h1gat[g][c * SHARD:(c + 1) * SHARD]
            m[f"ilo_{g}"], m[f"ihi_{g}"], m[f"ld_{g}"] = pl["lo"], pl["hi"], pl["ldst"]
            m[f"sv_{g}"] = _sval_stream(pl, s2[g])
            m[f"wst_{g}"] = pp[f"{g}_wst_2"]; m[f"b_{g}"] = pp[f"{g}_b_2"]
            m[f"Bw_{g}"] = pools[g][0][c]
        in_maps.append(m)
    t0 = time.time()
    resC = run_bass_kernel_spmd(ncC, in_maps, list(range(NCORES))).results
    walls["C"] = time.time() - t0

    c_parts = np.zeros((NCORES * 2, G, 512), np.float32)
    order = ["ast", "cfg", "dfg", "pdg"]
    for c in range(NCORES):
        for src_res, gl, slot in ((resB, ("ast", "dfg"), c),
                                  (resC, ("cfg", "pdg"), c + NCORES)):
            for g in gl:
                gb = pools[g][1][c]
                blk = order.index(g) * HID
                part = src_res[c][f"pool_{g}"]
                gs = min(P, G - gb)
                c_parts[slot, gb:gb + gs, blk:blk + HID] += part[0:gs]
    if "D" not in _cache:
        _cache["D"] = build_D()
    ncD, tD = _cache["D"]
    m = dict(c_parts=c_parts)
    for nm in ("se1", "se2", "cl1", "cl2", "cl3"):
        m[f"{nm}_w"] = pp[f"{nm}_w"]; m[f"{nm}_b"] = pp[f"{nm}_b"]
    t0 = time.time()
    resD = run_bass_kernel_spmd(ncD, [m] * NCORES, list(range(NCORES))).results
    walls["D"] = time.time() - t0
    kernel.last_walls = walls
    return np.ascontiguousarray(resD[0]["out"].astype(np.float32))
